# revision 2
# baseline (speedup 1.0000x reference)
"""CrossAttention kernel for 8 TRN2 NeuronCores.

Problem: B=8, N=M=1024, d_model=1024, 16 heads x 64 dim_head.
    q = x @ Wq; k = ctx @ Wk; v = ctx @ Wv   (per batch)
    out = softmax(q k^T / sqrt(64)) v @ Wo + bo

Sharding: data-parallel over batch. Core b computes batch element b end to
end; weights are replicated. No collectives.

Per-core dataflow (fp32 storage; float32r matmul compute = 4x fp32 PE rate,
measured end-to-end rel l2 err ~3.7e-4):
  stage 0: load x, ctx natural; PE-transpose 128x128 tiles -> xT, cT
           ([D on partitions, seq on free] - fp32 has no DMA transpose)
  stage 1a: qT = matmul(lhsT=Wq, rhs=xT) -> [INNER, N] layout, all four
           weight quarters up front (ACT-free PE prologue), plus
           quarter-0 k/v projections.
  stages 1b+2 interleaved by head-pair quarter: quarter wh's attention
           runs while quarter wh+1's k/v projection groups (and, in the
           last quarter, the Wo prefetch + early output chunks) are
           spread between its steps to keep PE dense while ACT does exp.
           Per step (head pair, N-chunk, M-chunk):
             sT pair = two row-tiled matmuls (heads at base partition
             0/64, K=64) into one [128,1024] PSUM tile
             pT = exp(SCALE*sT) - ONE [128,1024] ACT op for both heads
             (no max subtraction: scores are ~N(0,1), |s|<~6, safe)
             oT[65,512] += matmul(lhsT=v_h[128,65], rhs=pT half) over M,
             software-pipelined one step behind the sT/exp
           v carries an appended ones column ([M,65] per head) so row 64
           of oT accumulates the softmax denominator for free; the
           normalization reciprocal+copy run at block end, and the
           reciprocal broadcast (ones[1,64] matmul) + multiply are
           deferred one step to keep them off the PE critical path.
  stage 3: out = matmul(lhsT=oT, rhs=Wo) + bo (bias via a K=1 matmul
           with ones lhsT joining the same PSUM group); chunks that only
           need the first N half are emitted inside quarter 3.

Timing (8 cores, steady-state marginal per execution): ~413 us on HW;
TimelineSim predicts ~302 us (it does not model LDWEIGHTS).
"""

import numpy as np

import concourse.bass as bass
import concourse.mybir as mybir
import concourse.tile as tile
from concourse import bacc
from concourse import bass_utils
from concourse.masks import make_identity

P = 128
B = 8
N = 1024          # query length
M = 1024          # kv length
D = 1024          # d_model
H = 16
DH = 64
INNER = H * DH    # 1024
SCALE = DH ** -0.5
N_CORES = 8

F32 = mybir.dt.float32
MMDT = mybir.dt.float32r  # PE compute dtype: 4x fp32 throughput, ~1e-3 rel err


def _mm(nc, out, lhsT, rhs, start, stop):
    nc.tensor.matmul(out, lhsT, rhs, start=start, stop=stop)


def _build_body(tc, x_d, c_d, wq_d, wk_d, wv_d, wo_d, bo_d, out_d):
    nc = tc.nc
    EXP = mybir.ActivationFunctionType.Exp

    from contextlib import ExitStack
    ctx = ExitStack()

    const = ctx.enter_context(tc.tile_pool(name="const", bufs=1))
    ps_p = ctx.enter_context(tc.tile_pool(name="ps_p", bufs=2, space="PSUM"))
    ps_s = ctx.enter_context(tc.tile_pool(name="ps_s", bufs=2, space="PSUM"))
    ps_o = ctx.enter_context(tc.tile_pool(name="ps_o", bufs=2, space="PSUM"))
    # "big" slots (32KB/partition each): two slots cycle xn,cn -> qT,kT
    bigp = ctx.enter_context(tc.tile_pool(name="bigp", bufs=2))
    # xT slot reused for oT after stage 1; cT slot lives to end of stage 1
    xop = ctx.enter_context(tc.tile_pool(name="xop", bufs=1))
    ctp = ctx.enter_context(tc.tile_pool(name="ctp", bufs=1))
    wp = ctx.enter_context(tc.tile_pool(name="wp", bufs=2))
    wvp = ctx.enter_context(tc.tile_pool(name="wvp", bufs=1))
    outp = ctx.enter_context(tc.tile_pool(name="outp", bufs=1))
    vp = ctx.enter_context(tc.tile_pool(name="vp", bufs=1))
    pTp = ctx.enter_context(tc.tile_pool(name="pTp", bufs=2))

    ident = const.tile([P, P], F32, tag="ident")
    make_identity(nc, ident)
    ones = const.tile([1, P], MMDT, tag="ones")
    nc.vector.tensor_scalar(ones, ident[0:1, :], 0.0, 1.0,
                            mybir.AluOpType.mult, mybir.AluOpType.add)
    bo_sb = const.tile([1, D], MMDT, tag="bo")
    nc.sync.dma_start(bo_sb, bo_d.rearrange("(one d) -> one d", one=1))

    # ---- stage 0: transposed inputs xT [D, N], cT [D, M] --------------------
    xT = xop.tile([P, 8, N], MMDT, tag="xT")  # xT[pi, po, n] = x[n, po*128+pi]
    cT = ctp.tile([P, 8, M], MMDT, tag="cT")

    for src, dstT in ((x_d, xT), (c_d, cT)):
        nat = bigp.tile([P, 8, D], F32, tag="big",
                        name=f"nat_{src.name}")  # [seq%128, seq//128, d]
        src_r = src.rearrange("(so pi) d -> pi so d", pi=P)
        for so in range(8):
            if so == 0:
                # fine-grained first chunk: the very first transposes gate
                # the whole PE stream on this DMA
                for dq in range(4):
                    nc.sync.dma_start(nat[:, 0, dq * 256:(dq + 1) * 256],
                                      src_r[:, 0, dq * 256:(dq + 1) * 256])
            else:
                nc.sync.dma_start(nat[:, so, :], src_r[:, so, :])
            for dc in range(8):
                pst = ps_s.tile([P, 512], F32, tag="s")
                nc.tensor.transpose(
                    pst[:, :P], nat[:, so, dc * P:(dc + 1) * P], ident
                )
                nc.vector.tensor_copy(dstT[:, dc, so * P:(so + 1) * P], pst[:, :P])

    # ---- stage 1a: q projection + quarter-0 k/v (ACT-free PE prologue) ----
    qT = bigp.tile([P, 8, N], MMDT, tag="big",
                   name="qT")  # qT[pi, po, n] = q[n, po*128+pi]
    kT = bigp.tile([P, 8, M], MMDT, tag="big", name="kT")
    # v[pi, mo, h, 0:64] = v[mo*128+pi, h*64+:], col 64 = 1.0 (denominator)
    v = vp.tile([P, 8, H, DH + 1], MMDT, tag="v")
    # f32r memset fails ISA codegen; write the ones column as ident*0 + 1.0
    nc.vector.tensor_scalar(
        v[:, :, :, DH:DH + 1],
        ident.rearrange("p (a b c) -> p a b c", a=8, b=H, c=1),
        0.0, 1.0, mybir.AluOpType.mult, mybir.AluOpType.add)

    WQ = 256  # weight tile: quarter of INNER columns
    wq_r = wq_d.rearrange("(po pi) i -> pi po i", pi=P)
    wk_r = wk_d.rearrange("(po pi) i -> pi po i", pi=P)
    wv_r = wv_d.rearrange("(po pi) i -> pi po i", pi=P)

    def q_proj_quarter(wh):
        wt = wp.tile([P, 8, WQ], MMDT, tag="w", name=f"wq_{wh}")
        nc.sync.dma_start(wt, wq_r[:, :, wh * WQ:(wh + 1) * WQ])
        for ic in range(2):
            icg = wh * 2 + ic
            for nf in range(2):
                ps = ps_p.tile([P, 512], F32, tag="p", name="ps_q")
                for po in range(8):
                    _mm(nc, ps, wt[:, po, ic * P:(ic + 1) * P],
                        xT[:, po, nf * 512:(nf + 1) * 512],
                        start=(po == 0), stop=(po == 7))
                nc.vector.tensor_copy(qT[:, icg, nf * 512:(nf + 1) * 512], ps)

    def k_jobs(wh):
        """Emitter thunks for quarter wh's k projection (uses cT)."""
        wkt = wp.tile([P, 8, WQ], MMDT, tag="w", name=f"wk_{wh}")
        nc.sync.dma_start(wkt, wk_r[:, :, wh * WQ:(wh + 1) * WQ])

        def k_group(ic, nfk):
            icg = wh * 2 + ic
            ps = ps_p.tile([P, 512], F32, tag="p", name="ps_k")
            for po in range(8):
                _mm(nc, ps, wkt[:, po, ic * P:(ic + 1) * P],
                    cT[:, po, nfk * 512:(nfk + 1) * 512],
                    start=(po == 0), stop=(po == 7))
            nc.vector.tensor_copy(kT[:, icg, nfk * 512:(nfk + 1) * 512], ps)

        return [lambda ic=ic, nfk=nfk: k_group(ic, nfk)
                for ic in range(2) for nfk in range(2)]

    # v projection runs in HALVES (N=512 streams, half the matmul count of
    # the old per-quarter N=256 groups): half h covers heads 8h..8h+7.
    wv_tiles = {}

    def v_load(half):
        wvt = wvp.tile([P, 8, 512], MMDT, tag="wv", name=f"wv_{half}")
        nc.sync.dma_start(wvt, wv_r[:, :, half * 512:(half + 1) * 512])
        wv_tiles[half] = wvt

    def v_group(half, mc):
        ps = ps_p.tile([P, 512], F32, tag="p", name="ps_v")
        for po in range(8):
            _mm(nc, ps, cT[:, po, mc * P:(mc + 1) * P],
                wv_tiles[half][:, po, :],
                start=(po == 0), stop=(po == 7))
        nc.vector.tensor_copy(
            v[:, mc, half * 8:(half + 1) * 8, 0:DH],
            ps.rearrange("p (h dh) -> p h dh", dh=DH),
        )

    for wh in range(4):
        q_proj_quarter(wh)
    v_load(0)
    for job in k_jobs(0):
        job()
    for mc in range(8):
        v_group(0, mc)

    # ---- stage 2: attention, interleaved with next quarter's projections ----
    # Per quarter: 2 head pairs x 2 N-chunks x 8 M-chunks = 32 steps. Each
    # step: paired sT matmuls into one [128,1024] psum (row-tiled heads at
    # base partition 0/64), ONE exp over both heads, then the previous
    # step's o-accumulation matmuls (software pipeline: PE never waits on
    # the exp it just issued). Next quarter's k/v projection groups are
    # spread between steps to fill PE time while ACT churns exps.
    oT = xop.tile([P, 8, N], MMDT, tag="xT",
                  name="oT")  # oT[pi, po, n] = o[n, po*128+pi]

    def norm_a(ots, hp, nf):
        """Block end: reciprocal + copy out of PSUM (frees the o slots)."""
        rcp = pTp.tile([1, 1024], MMDT, tag="pT", name="rcp")
        for hi in range(2):
            rs = hi * DH
            with nc.allow_low_precision(reason="f32r softmax denom recip"):
                nc.vector.reciprocal(rcp[0:1, hi * 512:(hi + 1) * 512],
                                     ots[hi][DH:DH + 1, :])
            nc.vector.tensor_copy(
                oT[rs:rs + DH, hp, nf * 512:(nf + 1) * 512], ots[hi][0:DH, :])
        return rcp

    def norm_b(rcp, hp, nf):
        """Deferred one step: broadcast reciprocal on PE, multiply in place."""
        for hi in range(2):
            rs = hi * DH
            bc = ps_p.tile([P, 512], F32, tag="p", name="bc")
            _mm(nc, bc[0:DH, :], ones[0:1, 0:DH],
                rcp[0:1, hi * 512:(hi + 1) * 512], start=True, stop=True)
            oT_slice = oT[rs:rs + DH, hp, nf * 512:(nf + 1) * 512]
            nc.vector.tensor_mul(oT_slice, oT_slice, bc[0:DH, :])

    def emit_oT(ots, hp, nf, mc, pt):
        for hi in range(2):
            h = 2 * hp + hi
            _mm(nc, ots[hi][0:DH + 1, :], v[:, mc, h, :],
                pt[:, hi * 512:(hi + 1) * 512],
                start=(mc == 0), stop=(mc == 7))
        if mc == 7:
            return (norm_a(ots, hp, nf), hp, nf)
        return None

    wo_r = wo_d.rearrange("(po pi) d -> pi po d", pi=P)
    wo_box = {}

    def wo_load():
        # Full Wo into cT's slot: cT is dead once the carried k(icg=7)
        # groups have run. Two half DMAs so dh=0 chunks only wait on the
        # first half landing.
        wt = ctp.tile([P, 8, D], MMDT, tag="cT", name="wo_t")
        nc.sync.dma_start(wt[:, :, 0:512], wo_r[:, :, 0:512])
        nc.sync.dma_start(wt[:, :, 512:D], wo_r[:, :, 512:D])
        wo_box[0] = wt

    def out_chunk(dh, nc8):
        # N=512 output chunk (half the matmul count of the old N=256 ones).
        ps = ps_p.tile([P, 512], F32, tag="p", name="ps_out")
        for po in range(8):
            _mm(nc, ps, oT[:, po, nc8 * P:(nc8 + 1) * P],
                wo_box[0][:, po, dh * 512:(dh + 1) * 512],
                start=(po == 0), stop=False)
        _mm(nc, ps, ones[0:1, 0:P], bo_sb[0:1, dh * 512:(dh + 1) * 512],
            start=False, stop=True)
        ot = outp.tile([P, 512], F32, tag="out", name="ot")
        nc.vector.tensor_copy(ot, ps)
        nc.sync.dma_start(out_d[nc8 * P:(nc8 + 1) * P, dh * 512:(dh + 1) * 512],
                          ot)

    carry = []
    for wh in range(4):
        if wh < 3:
            if wh == 0:
                kv = k_jobs(1) + [lambda: v_load(1)] + \
                    [lambda mc=mc: v_group(1, mc) for mc in range(4)]
            elif wh == 1:
                kv = [lambda mc=mc: v_group(1, mc) for mc in range(4, 8)] + \
                    k_jobs(2)
            else:
                kv = k_jobs(3)
                # k(icg=7) chunks are first needed at quarter-3 step 16;
                # keep them as quarter-3 filler (its early blocks otherwise
                # have no projection work between attention steps)
                carry = kv[2:4]
                kv = kv[0:2]
            next_jobs = [((i + 1) / (len(kv) + 1), j) for i, j in enumerate(kv)]
        else:
            # quarter 3: carried k groups retire cT, then Wo loads into its
            # slot; chunks over n in [0,512) depend only on the nf=0 blocks,
            # whose last producer (pair 7, nf=0) ends at step 24/32 -- emit
            # them in the last quarter's tail.
            next_jobs = [(0.10, carry[0]), (0.35, carry[1]),
                         (0.45, wo_load)]
            next_jobs += [(0.79 + 0.04 * nc8, (lambda nc8=nc8: out_chunk(0, nc8)))
                          for nc8 in range(4)]
        steps = [(hp, nf, mc)
                 for hp in (2 * wh, 2 * wh + 1)
                 for nf in range(2)
                 for mc in range(8)]
        n_steps = len(steps)
        pending = None
        pending_norm = None
        ots_cur = None
        job_i = 0
        for si, (hp, nf, mc) in enumerate(steps):
            if mc == 0:
                ots_cur = [ps_o.tile([P, 512], F32, tag="o",
                                     name=f"ot_{wh}_{hp}_{nf}_{i}")
                           for i in range(2)]
            st = ps_s.tile([P, 1024], F32, tag="s", name="st")
            for hi in range(2):
                rs = hi * DH
                _mm(nc, st[:, hi * 512:(hi + 1) * 512],
                    kT[rs:rs + DH, hp, mc * P:(mc + 1) * P],
                    qT[rs:rs + DH, hp, nf * 512:(nf + 1) * 512],
                    start=True, stop=True)
            pt = pTp.tile([P, 1024], MMDT, tag="pT")
            nc.scalar.activation(pt, st, EXP, scale=SCALE)
            if pending_norm is not None:
                norm_b(*pending_norm)
                pending_norm = None
            while job_i < len(next_jobs) and \
                    next_jobs[job_i][0] * n_steps <= si + 1:
                next_jobs[job_i][1]()
                job_i += 1
            if pending is not None:
                pending_norm = emit_oT(*pending) or pending_norm
            pending = (ots_cur, hp, nf, mc, pt)
        pending_norm = emit_oT(*pending) or pending_norm
        if pending_norm is not None:
            norm_b(*pending_norm)
            pending_norm = None
        while job_i < len(next_jobs):
            next_jobs[job_i][1]()
            job_i += 1

    # ---- stage 3: remaining output chunks -----------------------------------
    for nc8 in range(4, 8):
        out_chunk(0, nc8)
    for nc8 in range(8):
        out_chunk(1, nc8)

    ctx.close()


_NC_CACHE = None


def build_nc():
    global _NC_CACHE
    if _NC_CACHE is not None:
        return _NC_CACHE
    nc = bacc.Bacc("TRN2", target_bir_lowering=False, debug=False,
                   num_devices=N_CORES)
    x_d = nc.dram_tensor("x", [N, D], F32, kind="ExternalInput").ap()
    c_d = nc.dram_tensor("context", [M, D], F32, kind="ExternalInput").ap()
    wq_d = nc.dram_tensor("Wq", [D, INNER], MMDT, kind="ExternalInput").ap()
    wk_d = nc.dram_tensor("Wk", [D, INNER], MMDT, kind="ExternalInput").ap()
    wv_d = nc.dram_tensor("Wv", [D, INNER], MMDT, kind="ExternalInput").ap()
    wo_d = nc.dram_tensor("Wo", [INNER, D], MMDT, kind="ExternalInput").ap()
    bo_d = nc.dram_tensor("bo", [D], MMDT, kind="ExternalInput").ap()
    out_d = nc.dram_tensor("out", [N, D], F32, kind="ExternalOutput").ap()

    with tile.TileContext(nc) as tc:
        _build_body(tc, x_d, c_d, wq_d, wk_d, wv_d, wo_d, bo_d, out_d)
    nc.compile()
    _NC_CACHE = nc
    return nc


def make_in_maps(x, context, Wq, Wk, Wv, Wo, bo):
    f = lambda a: np.ascontiguousarray(np.asarray(a, dtype=np.float32))
    x, context = f(x), f(context)
    Wq, Wk, Wv, Wo, bo = f(Wq), f(Wk), f(Wv), f(Wo), f(bo)
    return [
        {"x": x[b], "context": context[b], "Wq": Wq, "Wk": Wk, "Wv": Wv,
         "Wo": Wo, "bo": bo}
        for b in range(B)
    ]


def run(in_maps, trace=False, **kw):
    nc = build_nc()
    return bass_utils.run_bass_kernel_spmd(
        nc, in_maps, core_ids=list(range(N_CORES)), trace=trace, **kw)


def kernel(x, context, Wq, Wk, Wv, Wo, bo):
    res = run(make_in_maps(x, context, Wq, Wk, Wv, Wo, bo))
    return np.stack([res.results[b]["out"] for b in range(B)], axis=0)



# revision 3
# speedup vs baseline: 1.6238x; 1.6238x over previous
"""CrossAttention kernel for 8 TRN2 NeuronCores.

Problem: B=8, N=M=1024, d_model=1024, 16 heads x 64 dim_head.
    q = x @ Wq; k = ctx @ Wk; v = ctx @ Wv   (per batch)
    out = softmax(q k^T / sqrt(64)) v @ Wo + bo

Sharding: data-parallel over batch. Core b computes batch element b end to
end; weights are replicated. No collectives.

Per-core dataflow (fp32 storage; float32r matmul compute = 4x fp32 PE rate,
measured end-to-end rel l2 err ~3.7e-4):
  stage 0: load x, ctx natural; PE-transpose 128x128 tiles -> xT, cT
           ([D on partitions, seq on free] - fp32 has no DMA transpose)
  stage 1a: qT = matmul(lhsT=Wq, rhs=xT) -> [INNER, N] layout, all four
           weight quarters up front (ACT-free PE prologue), plus
           quarter-0 k/v projections.
  stages 1b+2 interleaved by head-pair quarter: quarter wh's attention
           runs while quarter wh+1's k/v projection groups (and, in the
           last quarter, the Wo prefetch + early output chunks) are
           spread between its steps to keep PE dense while ACT does exp.
           Per step (head pair, N-chunk, M-chunk):
             sT pair = two row-tiled matmuls (heads at base partition
             0/64, K=64) into one [128,1024] PSUM tile
             pT = exp(SCALE*sT) - ONE [128,1024] ACT op for both heads
             (no max subtraction: scores are ~N(0,1), |s|<~6, safe)
             oT[65,512] += matmul(lhsT=v_h[128,65], rhs=pT half) over M,
             software-pipelined one step behind the sT/exp
           v carries an appended ones column ([M,65] per head) so row 64
           of oT accumulates the softmax denominator for free; the
           normalization reciprocal+copy run at block end, and the
           reciprocal broadcast (ones[1,64] matmul) + multiply are
           deferred one step to keep them off the PE critical path.
  stage 3: out = matmul(lhsT=oT, rhs=Wo) + bo (bias via a K=1 matmul
           with ones lhsT joining the same PSUM group); chunks that only
           need the first N half are emitted inside quarter 3.

Instruction-count reductions vs the first working version (HW per-matmul
overhead for distinct-weight f32r is ~100-200ns, so narrow streams lose):
  - v projection in HALVES (N=512 streams, 128 matmuls) instead of
    quarters (N=256, 256 matmuls); half 0 runs in stage 1a, half 1 as
    quarter-0/1 filler.
  - full Wo loads into cT's SBUF slot once the carried k(icg=7) groups
    retire cT (two half-DMAs so dh=0 output chunks only wait the first);
    output chunks are N=512 (128 matmuls vs 256+32).
  - rcp tiles share the pTp pool slots (SBUF is exactly full otherwise).

Timing (8 cores, steady-state marginal per execution, min over 6 noisy
rounds): ~500 us on HW vs ~651 us for the previous version measured the
same way. Known-failed variants (walrus backend crash, do not retry
blindly): q-proj N=1024 groups through ps_s slots; 4-transposes-per-PSUM
-tile batched evacuation.
"""

import numpy as np

import concourse.bass as bass
import concourse.mybir as mybir
import concourse.tile as tile
from concourse import bacc
from concourse import bass_utils
from concourse.masks import make_identity

P = 128
B = 8
N = 1024          # query length
M = 1024          # kv length
D = 1024          # d_model
H = 16
DH = 64
INNER = H * DH    # 1024
SCALE = DH ** -0.5
N_CORES = 8

F32 = mybir.dt.float32
MMDT = mybir.dt.float32r  # PE compute dtype: 4x fp32 throughput, ~1e-3 rel err


def _mm(nc, out, lhsT, rhs, start, stop):
    nc.tensor.matmul(out, lhsT, rhs, start=start, stop=stop)


def _build_body(tc, x_d, c_d, wq_d, wk_d, wv_d, wo_d, bo_d, out_d):
    nc = tc.nc
    EXP = mybir.ActivationFunctionType.Exp

    from contextlib import ExitStack
    ctx = ExitStack()

    const = ctx.enter_context(tc.tile_pool(name="const", bufs=1))
    ps_p = ctx.enter_context(tc.tile_pool(name="ps_p", bufs=2, space="PSUM"))
    ps_s = ctx.enter_context(tc.tile_pool(name="ps_s", bufs=2, space="PSUM"))
    ps_o = ctx.enter_context(tc.tile_pool(name="ps_o", bufs=2, space="PSUM"))
    # "big" slots (32KB/partition each): two slots cycle xn,cn -> qT,kT
    bigp = ctx.enter_context(tc.tile_pool(name="bigp", bufs=2))
    # xT slot reused for oT after stage 1; cT slot lives to end of stage 1
    xop = ctx.enter_context(tc.tile_pool(name="xop", bufs=1))
    ctp = ctx.enter_context(tc.tile_pool(name="ctp", bufs=1))
    wp = ctx.enter_context(tc.tile_pool(name="wp", bufs=2))
    wvp = ctx.enter_context(tc.tile_pool(name="wvp", bufs=1))
    outp = ctx.enter_context(tc.tile_pool(name="outp", bufs=1))
    vp = ctx.enter_context(tc.tile_pool(name="vp", bufs=1))
    pTp = ctx.enter_context(tc.tile_pool(name="pTp", bufs=2))

    ident = const.tile([P, P], F32, tag="ident")
    make_identity(nc, ident)
    ones = const.tile([1, P], MMDT, tag="ones")
    nc.vector.tensor_scalar(ones, ident[0:1, :], 0.0, 1.0,
                            mybir.AluOpType.mult, mybir.AluOpType.add)
    bo_sb = const.tile([1, D], MMDT, tag="bo")
    nc.sync.dma_start(bo_sb, bo_d.rearrange("(one d) -> one d", one=1))

    # ---- stage 0: transposed inputs xT [D, N], cT [D, M] --------------------
    xT = xop.tile([P, 8, N], MMDT, tag="xT")  # xT[pi, po, n] = x[n, po*128+pi]
    cT = ctp.tile([P, 8, M], MMDT, tag="cT")

    for src, dstT in ((x_d, xT), (c_d, cT)):
        nat = bigp.tile([P, 8, D], F32, tag="big",
                        name=f"nat_{src.name}")  # [seq%128, seq//128, d]
        src_r = src.rearrange("(so pi) d -> pi so d", pi=P)
        for so in range(8):
            if so == 0:
                # fine-grained first chunk: the very first transposes gate
                # the whole PE stream on this DMA
                for dq in range(4):
                    nc.sync.dma_start(nat[:, 0, dq * 256:(dq + 1) * 256],
                                      src_r[:, 0, dq * 256:(dq + 1) * 256])
            else:
                nc.sync.dma_start(nat[:, so, :], src_r[:, so, :])
            for dc in range(8):
                pst = ps_s.tile([P, 512], F32, tag="s")
                nc.tensor.transpose(
                    pst[:, :P], nat[:, so, dc * P:(dc + 1) * P], ident
                )
                nc.vector.tensor_copy(dstT[:, dc, so * P:(so + 1) * P], pst[:, :P])

    # ---- stage 1a: q projection + quarter-0 k/v (ACT-free PE prologue) ----
    qT = bigp.tile([P, 8, N], MMDT, tag="big",
                   name="qT")  # qT[pi, po, n] = q[n, po*128+pi]
    kT = bigp.tile([P, 8, M], MMDT, tag="big", name="kT")
    # v[pi, mo, h, 0:64] = v[mo*128+pi, h*64+:], col 64 = 1.0 (denominator)
    v = vp.tile([P, 8, H, DH + 1], MMDT, tag="v")
    # f32r memset fails ISA codegen; write the ones column as ident*0 + 1.0
    nc.vector.tensor_scalar(
        v[:, :, :, DH:DH + 1],
        ident.rearrange("p (a b c) -> p a b c", a=8, b=H, c=1),
        0.0, 1.0, mybir.AluOpType.mult, mybir.AluOpType.add)

    WQ = 256  # weight tile: quarter of INNER columns
    wq_r = wq_d.rearrange("(po pi) i -> pi po i", pi=P)
    wk_r = wk_d.rearrange("(po pi) i -> pi po i", pi=P)
    wv_r = wv_d.rearrange("(po pi) i -> pi po i", pi=P)

    def q_proj_quarter(wh):
        wt = wp.tile([P, 8, WQ], MMDT, tag="w", name=f"wq_{wh}")
        nc.sync.dma_start(wt, wq_r[:, :, wh * WQ:(wh + 1) * WQ])
        for ic in range(2):
            icg = wh * 2 + ic
            for nf in range(2):
                ps = ps_p.tile([P, 512], F32, tag="p", name="ps_q")
                for po in range(8):
                    _mm(nc, ps, wt[:, po, ic * P:(ic + 1) * P],
                        xT[:, po, nf * 512:(nf + 1) * 512],
                        start=(po == 0), stop=(po == 7))
                nc.vector.tensor_copy(qT[:, icg, nf * 512:(nf + 1) * 512], ps)

    def k_jobs(wh):
        """Emitter thunks for quarter wh's k projection (uses cT)."""
        wkt = wp.tile([P, 8, WQ], MMDT, tag="w", name=f"wk_{wh}")
        nc.sync.dma_start(wkt, wk_r[:, :, wh * WQ:(wh + 1) * WQ])

        def k_group(ic, nfk):
            icg = wh * 2 + ic
            ps = ps_p.tile([P, 512], F32, tag="p", name="ps_k")
            for po in range(8):
                _mm(nc, ps, wkt[:, po, ic * P:(ic + 1) * P],
                    cT[:, po, nfk * 512:(nfk + 1) * 512],
                    start=(po == 0), stop=(po == 7))
            nc.vector.tensor_copy(kT[:, icg, nfk * 512:(nfk + 1) * 512], ps)

        return [lambda ic=ic, nfk=nfk: k_group(ic, nfk)
                for ic in range(2) for nfk in range(2)]

    # v projection runs in HALVES (N=512 streams, half the matmul count of
    # the old per-quarter N=256 groups): half h covers heads 8h..8h+7.
    wv_tiles = {}

    def v_load(half):
        wvt = wvp.tile([P, 8, 512], MMDT, tag="wv", name=f"wv_{half}")
        nc.sync.dma_start(wvt, wv_r[:, :, half * 512:(half + 1) * 512])
        wv_tiles[half] = wvt

    def v_group(half, mc):
        ps = ps_p.tile([P, 512], F32, tag="p", name="ps_v")
        for po in range(8):
            _mm(nc, ps, cT[:, po, mc * P:(mc + 1) * P],
                wv_tiles[half][:, po, :],
                start=(po == 0), stop=(po == 7))
        nc.vector.tensor_copy(
            v[:, mc, half * 8:(half + 1) * 8, 0:DH],
            ps.rearrange("p (h dh) -> p h dh", dh=DH),
        )

    for wh in range(4):
        q_proj_quarter(wh)
    v_load(0)
    for job in k_jobs(0):
        job()
    for mc in range(8):
        v_group(0, mc)

    # ---- stage 2: attention, interleaved with next quarter's projections ----
    # Per quarter: 2 head pairs x 2 N-chunks x 8 M-chunks = 32 steps. Each
    # step: paired sT matmuls into one [128,1024] psum (row-tiled heads at
    # base partition 0/64), ONE exp over both heads, then the previous
    # step's o-accumulation matmuls (software pipeline: PE never waits on
    # the exp it just issued). Next quarter's k/v projection groups are
    # spread between steps to fill PE time while ACT churns exps.
    oT = xop.tile([P, 8, N], MMDT, tag="xT",
                  name="oT")  # oT[pi, po, n] = o[n, po*128+pi]

    def norm_a(ots, hp, nf):
        """Block end: reciprocal + copy out of PSUM (frees the o slots)."""
        rcp = pTp.tile([1, 1024], MMDT, tag="pT", name="rcp")
        for hi in range(2):
            rs = hi * DH
            with nc.allow_low_precision(reason="f32r softmax denom recip"):
                nc.vector.reciprocal(rcp[0:1, hi * 512:(hi + 1) * 512],
                                     ots[hi][DH:DH + 1, :])
            nc.vector.tensor_copy(
                oT[rs:rs + DH, hp, nf * 512:(nf + 1) * 512], ots[hi][0:DH, :])
        return rcp

    def norm_b(rcp, hp, nf):
        """Deferred one step: broadcast reciprocal on PE, multiply in place."""
        for hi in range(2):
            rs = hi * DH
            bc = ps_p.tile([P, 512], F32, tag="p", name="bc")
            _mm(nc, bc[0:DH, :], ones[0:1, 0:DH],
                rcp[0:1, hi * 512:(hi + 1) * 512], start=True, stop=True)
            oT_slice = oT[rs:rs + DH, hp, nf * 512:(nf + 1) * 512]
            nc.vector.tensor_mul(oT_slice, oT_slice, bc[0:DH, :])

    def emit_oT(ots, hp, nf, mc, pt):
        for hi in range(2):
            h = 2 * hp + hi
            _mm(nc, ots[hi][0:DH + 1, :], v[:, mc, h, :],
                pt[:, hi * 512:(hi + 1) * 512],
                start=(mc == 0), stop=(mc == 7))
        if mc == 7:
            return (norm_a(ots, hp, nf), hp, nf)
        return None

    wo_r = wo_d.rearrange("(po pi) d -> pi po d", pi=P)
    wo_box = {}

    def wo_load():
        # Full Wo into cT's slot: cT is dead once the carried k(icg=7)
        # groups have run. Two half DMAs so dh=0 chunks only wait on the
        # first half landing.
        wt = ctp.tile([P, 8, D], MMDT, tag="cT", name="wo_t")
        nc.sync.dma_start(wt[:, :, 0:512], wo_r[:, :, 0:512])
        nc.sync.dma_start(wt[:, :, 512:D], wo_r[:, :, 512:D])
        wo_box[0] = wt

    def out_chunk(dh, nc8):
        # N=512 output chunk (half the matmul count of the old N=256 ones).
        ps = ps_p.tile([P, 512], F32, tag="p", name="ps_out")
        for po in range(8):
            _mm(nc, ps, oT[:, po, nc8 * P:(nc8 + 1) * P],
                wo_box[0][:, po, dh * 512:(dh + 1) * 512],
                start=(po == 0), stop=False)
        _mm(nc, ps, ones[0:1, 0:P], bo_sb[0:1, dh * 512:(dh + 1) * 512],
            start=False, stop=True)
        ot = outp.tile([P, 512], F32, tag="out", name="ot")
        nc.vector.tensor_copy(ot, ps)
        nc.sync.dma_start(out_d[nc8 * P:(nc8 + 1) * P, dh * 512:(dh + 1) * 512],
                          ot)

    carry = []
    for wh in range(4):
        if wh < 3:
            if wh == 0:
                kv = k_jobs(1) + [lambda: v_load(1)] + \
                    [lambda mc=mc: v_group(1, mc) for mc in range(4)]
            elif wh == 1:
                kv = [lambda mc=mc: v_group(1, mc) for mc in range(4, 8)] + \
                    k_jobs(2)
            else:
                kv = k_jobs(3)
                # k(icg=7) chunks are first needed at quarter-3 step 16;
                # keep them as quarter-3 filler (its early blocks otherwise
                # have no projection work between attention steps)
                carry = kv[2:4]
                kv = kv[0:2]
            next_jobs = [((i + 1) / (len(kv) + 1), j) for i, j in enumerate(kv)]
        else:
            # quarter 3: carried k groups retire cT, then Wo loads into its
            # slot; chunks over n in [0,512) depend only on the nf=0 blocks,
            # whose last producer (pair 7, nf=0) ends at step 24/32 -- emit
            # them in the last quarter's tail.
            next_jobs = [(0.10, carry[0]), (0.35, carry[1]),
                         (0.45, wo_load)]
            next_jobs += [(0.79 + 0.04 * nc8, (lambda nc8=nc8: out_chunk(0, nc8)))
                          for nc8 in range(4)]
        steps = [(hp, nf, mc)
                 for hp in (2 * wh, 2 * wh + 1)
                 for nf in range(2)
                 for mc in range(8)]
        n_steps = len(steps)
        pending = None
        pending_norm = None
        ots_cur = None
        job_i = 0
        for si, (hp, nf, mc) in enumerate(steps):
            if mc == 0:
                ots_cur = [ps_o.tile([P, 512], F32, tag="o",
                                     name=f"ot_{wh}_{hp}_{nf}_{i}")
                           for i in range(2)]
            st = ps_s.tile([P, 1024], F32, tag="s", name="st")
            for hi in range(2):
                rs = hi * DH
                _mm(nc, st[:, hi * 512:(hi + 1) * 512],
                    kT[rs:rs + DH, hp, mc * P:(mc + 1) * P],
                    qT[rs:rs + DH, hp, nf * 512:(nf + 1) * 512],
                    start=True, stop=True)
            pt = pTp.tile([P, 1024], MMDT, tag="pT")
            nc.scalar.activation(pt, st, EXP, scale=SCALE)
            if pending_norm is not None:
                norm_b(*pending_norm)
                pending_norm = None
            while job_i < len(next_jobs) and \
                    next_jobs[job_i][0] * n_steps <= si + 1:
                next_jobs[job_i][1]()
                job_i += 1
            if pending is not None:
                pending_norm = emit_oT(*pending) or pending_norm
            pending = (ots_cur, hp, nf, mc, pt)
        pending_norm = emit_oT(*pending) or pending_norm
        if pending_norm is not None:
            norm_b(*pending_norm)
            pending_norm = None
        while job_i < len(next_jobs):
            next_jobs[job_i][1]()
            job_i += 1

    # ---- stage 3: remaining output chunks -----------------------------------
    for nc8 in range(4, 8):
        out_chunk(0, nc8)
    for nc8 in range(8):
        out_chunk(1, nc8)

    ctx.close()


_NC_CACHE = None


def build_nc():
    global _NC_CACHE
    if _NC_CACHE is not None:
        return _NC_CACHE
    nc = bacc.Bacc("TRN2", target_bir_lowering=False, debug=False,
                   num_devices=N_CORES)
    x_d = nc.dram_tensor("x", [N, D], F32, kind="ExternalInput").ap()
    c_d = nc.dram_tensor("context", [M, D], F32, kind="ExternalInput").ap()
    wq_d = nc.dram_tensor("Wq", [D, INNER], MMDT, kind="ExternalInput").ap()
    wk_d = nc.dram_tensor("Wk", [D, INNER], MMDT, kind="ExternalInput").ap()
    wv_d = nc.dram_tensor("Wv", [D, INNER], MMDT, kind="ExternalInput").ap()
    wo_d = nc.dram_tensor("Wo", [INNER, D], MMDT, kind="ExternalInput").ap()
    bo_d = nc.dram_tensor("bo", [D], MMDT, kind="ExternalInput").ap()
    out_d = nc.dram_tensor("out", [N, D], F32, kind="ExternalOutput").ap()

    with tile.TileContext(nc) as tc:
        _build_body(tc, x_d, c_d, wq_d, wk_d, wv_d, wo_d, bo_d, out_d)
    nc.compile()
    _NC_CACHE = nc
    return nc


def make_in_maps(x, context, Wq, Wk, Wv, Wo, bo):
    f = lambda a: np.ascontiguousarray(np.asarray(a, dtype=np.float32))
    x, context = f(x), f(context)
    Wq, Wk, Wv, Wo, bo = f(Wq), f(Wk), f(Wv), f(Wo), f(bo)
    return [
        {"x": x[b], "context": context[b], "Wq": Wq, "Wk": Wk, "Wv": Wv,
         "Wo": Wo, "bo": bo}
        for b in range(B)
    ]


def run(in_maps, trace=False, **kw):
    nc = build_nc()
    return bass_utils.run_bass_kernel_spmd(
        nc, in_maps, core_ids=list(range(N_CORES)), trace=trace, **kw)


def kernel(x, context, Wq, Wk, Wv, Wo, bo):
    res = run(make_in_maps(x, context, Wq, Wk, Wv, Wo, bo))
    return np.stack([res.results[b]["out"] for b in range(B)], axis=0)

